# revision 53
# baseline (speedup 1.0000x reference)
"""Trainium2 Bass kernel for nn_Explore_decoder_add (histogram_binning).

Strategy (8 NeuronCores, tensor-parallel on vocab), v9:
  - Wec split: W0 (h_t half) stays bf16 (precision-critical: h ~ N(0,1));
    W1 (c_s half) is fp8e4 x4 (its logit contribution is ~15x smaller).
    Measured end-to-end rel err ~5e-3 vs the 2e-2 gate.
  - The seen-id penalty (histogram) and the softmax normalization are
    applied on the HOST: the kernel returns raw exp(logits) per shard
    (fp16); the host zeroes the <=3200 masked entries, sums, normalizes.
  - Additive-attention bias fold: the host solves Wq^T d_b = k_b + bias
    (k_b = Wk^T x_b0 + bq + bk) and ships x' = x + d_b in fp16, so
    tanh(Wq^T x'_bs) == tanh(q + k) exactly.  The pooling q-path is then
    just 8 merged [128,400] fp16 matmuls + 8 constant-bias tanh
    activations over two batches each -- no per-batch bias plumbing on
    the critical path at all.
  - c_s still uses the ORIGINAL x (fp8 x16, s-on-partitions layout);
    scores -> fp8 exp -> PE-summed denominator (ones column holds 64 so
    v8 = csT/(64*ssum) = c_s_normalized/4, cancelling the x4 on W1).
  - bec is added mid-stream per PSUM bank by a DVE broadcast add
    (stride-0 free-dim AP) -- off the epilogue critical path.
  - DMA: sync ring carries packb/xqh/w0 g0..g3/w18 g0..g3 in consumption
    order (W1 groups last; pooling finishes before they land); gpsimd
    ring carries becp/xs in parallel.  Four per-bank fp16 output
    tensors, contiguous in DRAM, written from scalar/gpsimd/sync rings
    as each bank closes.
  - tile_wait_until hints push all bank matmuls after the pooling chain
    in the scheduler's static order (runtime stays semaphore-driven),
    keeping the tanh cadence ACT-bound instead of PE-bound.
"""

import numpy as np
import ml_dtypes

B, S, D = 16, 200, 128
V = 100000
NCORES = 8
VS = V // NCORES            # 12500 vocab per core
NCHUNK = 98                 # 98 chunks of 128
VSP = NCHUNK * 128          # 12544 padded shard width
BANKS = (25, 25, 25, 23)    # chunks per PSUM bank (sum = 98)
SX = 16.0                   # fp8 scale on x (c_s path)
SW1 = 4.0                   # fp8 scale on W1
SK = 64.0                   # ssum ones value (= SX * SW1)
XQ_COLS = D + B * S         # wq columns + x' columns in the xqh tensor

_prog_cache = {}


def _build_program():
    import concourse.bacc as bacc
    import concourse.mybir as mybir
    import concourse.tile as tile

    f32 = mybir.dt.float32
    f16 = mybir.dt.float16
    bf16 = mybir.dt.bfloat16
    f8 = mybir.dt.float8e4
    OP = mybir.AluOpType
    ACT = mybir.ActivationFunctionType

    nc = bacc.Bacc("TRN2", target_bir_lowering=False, debug=False,
                   num_devices=NCORES)

    # ---- I/O -------------------------------------------------------------
    packb = nc.dram_tensor("packb", (D, B + 1), bf16,
                           kind="ExternalInput").ap()
    xqh = nc.dram_tensor("xqh", (D, XQ_COLS), f16, kind="ExternalInput").ap()
    xs0 = nc.dram_tensor("xs0", (128, B, D), f8, kind="ExternalInput").ap()
    xs1 = nc.dram_tensor("xs1", (72, B, D), f8, kind="ExternalInput").ap()
    w0 = nc.dram_tensor("w0", (D, VSP), bf16, kind="ExternalInput").ap()
    w18 = nc.dram_tensor("w18", (D, VSP), f8, kind="ExternalInput").ap()
    becp = nc.dram_tensor("becp", (128, NCHUNK), bf16,
                          kind="ExternalInput").ap()
    outs = [nc.dram_tensor(f"out{g}", (128, BANKS[g] * B), f16,
                           kind="ExternalOutput").ap() for g in range(4)]

    with tile.TileContext(nc) as tc:
        with (
            tc.tile_pool(name="sb", bufs=1) as sb,
            tc.tile_pool(name="pq", bufs=1, space="PSUM") as pq,
            tc.tile_pool(name="pp", bufs=1, space="PSUM") as pp,
        ):
            # ---- sync ring: packb, xqh, then w0/w18 in bank order --------
            packb_sb = sb.tile([D, B + 1], bf16, name="packb_sb")
            nc.sync.dma_start(out=packb_sb[:, :], in_=packb[:, :])
            x0T_sb = packb_sb[:, 0:B]
            wv_sb = packb_sb[:, B:B + 1]

            xqh_sb = sb.tile([D, XQ_COLS], f16, name="xqh_sb")
            cuts = (0, 528, 1328, XQ_COLS)
            for i in range(3):
                nc.sync.dma_start(out=xqh_sb[:, cuts[i]:cuts[i + 1]],
                                  in_=xqh[:, cuts[i]:cuts[i + 1]])
            wqh_sb = xqh_sb[:, 0:D]

            w0_sb = sb.tile([D, VSP], bf16, name="w0_sb")
            w18_sb = sb.tile([D, VSP], f8, name="w18_sb")
            for g in range(4):
                c0 = sum(BANKS[:g]) * 128
                c1 = c0 + BANKS[g] * 128
                nc.sync.dma_start(out=w0_sb[:, c0:c1], in_=w0[:, c0:c1])
            for g in range(4):
                c0 = sum(BANKS[:g]) * 128
                c1 = c0 + BANKS[g] * 128
                nc.sync.dma_start(out=w18_sb[:, c0:c1], in_=w18[:, c0:c1])

            # ---- gpsimd ring: becp, xs (parallel with sync ring) ---------
            becp2_sb = sb.tile([128, NCHUNK], bf16, name="becp2_sb")
            nc.gpsimd.dma_start(out=becp2_sb[:, :], in_=becp[:, :])
            xs0_sb = sb.tile([128, B, D], f8, name="xs0_sb")
            nc.gpsimd.dma_start(out=xs0_sb[:, :, :], in_=xs0[:, :, :])
            xs1_sb = sb.tile([128, B, D], f8, name="xs1_sb")
            nc.gpsimd.dma_start(out=xs1_sb[0:72, :, :], in_=xs1[:, :, :])

            # ---- constants ----------------------------------------------
            ones64 = sb.tile([128, 1], f8, name="ones64")
            nc.gpsimd.memset(ones64[:, :], SK)
            ones_row = sb.tile([1, 128], f32, name="ones_row")
            nc.gpsimd.memset(ones_row[:, :], 1.0)

            # ---- pooling chain (critical path; emitted first) ------------
            pmiscA = pp.tile([128, 512], f32, name="pmiscA", tag="miscA")
            pmiscB = pp.tile([128, 512], f32, name="pmiscB", tag="miscB")
            fT = sb.tile([128, B, S], bf16, name="fT")
            scT0 = pmiscA[:, 0:B]
            scT1 = pmiscA[0:72, 2 * B:3 * B]
            qps = [pq.tile([128, 2, S], f32, name=f"qps{i}", tag=f"q{i}")
                   for i in range(2)]
            # W0 terms are emitted in ~12-chunk slices interleaved with the
            # q/tanh pipeline: keeps the PE duty cycle high (full p-state)
            # and finishes W0 by the end of pooling.
            ps = [pp.tile([128, 32, B], f32, name=f"ps{g}", tag=f"ps{g}")
                  for g in range(4)]

            def bank_of(c):
                t = 0
                for g in range(4):
                    if c < t + BANKS[g]:
                        return g, c - t, t
                    t += BANKS[g]
                raise AssertionError

            def emit_w0(c0, c1):
                for c in range(c0, min(c1, NCHUNK)):
                    g, cl, _ = bank_of(c)
                    nc.tensor.matmul(
                        out=ps[g][:, cl, :],
                        lhsT=w0_sb[:, c * 128:(c + 1) * 128],
                        rhs=x0T_sb, start=(cl == 0), stop=False)

            for g2 in range(8):
                b = 2 * g2
                tile_i = qps[g2 % 2]
                nc.tensor.matmul(
                    out=tile_i[:, :, :], lhsT=wqh_sb,
                    rhs=xqh_sb[:, D + b * S:D + (b + 2) * S],
                    start=True, stop=True)
                nc.scalar.activation(out=fT[:, b:b + 2, :],
                                     in_=tile_i[:, :, :], func=ACT.Tanh)
                emit_w0(g2 * 13, (g2 + 1) * 13)
            # mid-stream bec broadcast-adds (DVE), per closed-W0 bank
            t = 0
            for g in range(4):
                nb = BANKS[g]
                nc.vector.tensor_tensor(
                    out=ps[g][:, 0:nb, :], in0=ps[g][:, 0:nb, :],
                    in1=becp2_sb[:, t:t + nb].unsqueeze(2)
                        .broadcast_to([128, nb, B]),
                    op=OP.add)
                t += nb
            for b in range(B):
                nc.tensor.matmul(out=scT0[:, b:b + 1],
                                 lhsT=fT[:, b, 0:128], rhs=wv_sb,
                                 start=(b == 0), stop=(b == B - 1))
                nc.tensor.matmul(out=scT1[:, b:b + 1],
                                 lhsT=fT[:, b, 128:200], rhs=wv_sb,
                                 start=(b == 0), stop=(b == B - 1))
            e8_0 = sb.tile([128, B], f8, name="e8_0")
            nc.scalar.activation(out=e8_0[:, :], in_=scT0, func=ACT.Exp)
            e8_1 = sb.tile([128, B], f8, name="e8_1")
            nc.scalar.activation(out=e8_1[0:72, :], in_=scT1, func=ACT.Exp)
            ssum_ps = pmiscB[0:1, 0:B]
            nc.tensor.matmul(out=ssum_ps, lhsT=ones64[:, :],
                             rhs=e8_0[:, :], start=True, stop=False)
            nc.tensor.matmul(out=ssum_ps, lhsT=ones64[0:72, :],
                             rhs=e8_1[0:72, :], start=False, stop=True)
            sinv_row = sb.tile([1, B], f32, name="sinv_row")
            nc.vector.reciprocal(sinv_row[:, :], ssum_ps)

            csT = pmiscB[:, 2 * B:3 * B]
            for b in range(B):
                nc.tensor.matmul(out=csT[:, b:b + 1], lhsT=xs0_sb[:, b, :],
                                 rhs=e8_0[:, b:b + 1], start=(b == 0),
                                 stop=False)
                nc.tensor.matmul(out=csT[:, b:b + 1],
                                 lhsT=xs1_sb[0:72, b, :],
                                 rhs=e8_1[0:72, b:b + 1], start=False,
                                 stop=(b == B - 1))
            sinv_ps = pmiscB[:, 3 * B:4 * B]
            nc.tensor.matmul(out=sinv_ps, lhsT=ones_row[0:1, :],
                             rhs=sinv_row[:, :], start=True, stop=True)
            sinv_sb = sb.tile([128, B], f32, name="sinv_sb")
            nc.vector.tensor_copy(sinv_sb[:, :], sinv_ps)
            v8 = sb.tile([128, B], f8, name="v8")
            nc.vector.tensor_tensor(out=v8[:, :], in0=csT,
                                    in1=sinv_sb[:, :], op=OP.mult)

            # W1 terms close each bank; exp; per-bank output DMA.
            out_dma = [nc.sync, nc.gpsimd, nc.gpsimd, nc.sync]
            t = 0
            for g in range(4):
                nb = BANKS[g]
                with tc.tile_wait_until(0.21 + 0.0005 * g):
                    for cl in range(nb):
                        c = t + cl
                        nc.tensor.matmul(
                            out=ps[g][:, cl, :],
                            lhsT=w18_sb[:, c * 128:(c + 1) * 128],
                            rhs=v8[:, :], start=False,
                            stop=(cl == nb - 1))
                    exp_g = sb.tile([128, nb, B], f16, name=f"exp{g}")
                    outr = outs[g].rearrange("p (c b) -> p c b", b=B)
                    if g >= 2:
                        # last banks: halve exp+out so the output DMA
                        # overlaps the tail W1 matmuls
                        h = nb // 2
                        nc.scalar.activation(out=exp_g[:, 0:h, :],
                                             in_=ps[g][:, 0:h, :],
                                             func=ACT.Exp)
                        out_dma[g].dma_start(out=outr[:, 0:h, :],
                                             in_=exp_g[:, 0:h, :])
                        nc.scalar.activation(out=exp_g[:, h:nb, :],
                                             in_=ps[g][:, h:nb, :],
                                             func=ACT.Exp)
                        out_dma[g].dma_start(out=outr[:, h:nb, :],
                                             in_=exp_g[:, h:nb, :])
                    else:
                        nc.scalar.activation(out=exp_g[:, :, :],
                                             in_=ps[g][:, 0:nb, :],
                                             func=ACT.Exp)
                        out_dma[g].dma_start(out=outr[:, :, :],
                                             in_=exp_g[:, :, :])
                t += nb

    nc.compile()
    return nc


def _get_program():
    if "nc" not in _prog_cache:
        _prog_cache["nc"] = _build_program()
    return _prog_cache["nc"]


def _host_inputs(x, x_ids, Wq, bq, Wk, bk, Wv, bv, Wec, bec):
    """Shared + per-core input arrays (host re-encodes layouts and folds
    the additive-attention bias into x via a 128x128 solve)."""
    bf = ml_dtypes.bfloat16
    f8 = ml_dtypes.float8_e4m3
    x = np.asarray(x, dtype=np.float32)
    x8 = (x * SX).astype(f8)                       # (B,S,D), c_s path
    # fold k_b + bias into the q path: x' = x + Wq^-T (k_b + bq + bk)
    Wq64 = np.asarray(Wq, np.float64)
    k_host = (x[:, 0, :].astype(np.float64) @ np.asarray(Wk, np.float64)
              + np.asarray(bq, np.float64) + np.asarray(bk, np.float64))
    delta = np.linalg.solve(Wq64.T, k_host.T).T    # (B, D)
    xp = x + delta[:, None, :].astype(np.float32)  # (B,S,D)
    xq = np.empty((D, XQ_COLS), np.float16)
    xq[:, 0:D] = np.asarray(Wq, np.float32).astype(np.float16)
    xq[:, D:] = xp.transpose(2, 0, 1).reshape(D, B * S).astype(np.float16)
    packb = np.concatenate([
        np.ascontiguousarray(x[:, 0, :].T.astype(bf)),
        np.asarray(Wv, np.float32).astype(bf),
    ], axis=1)
    shared = {
        "packb": np.ascontiguousarray(packb),
        "xqh": np.ascontiguousarray(xq),
        "xs0": np.ascontiguousarray(x8[:, 0:128, :].transpose(1, 0, 2)),
        "xs1": np.ascontiguousarray(x8[:, 128:200, :].transpose(1, 0, 2)),
    }
    Wec = np.asarray(Wec, np.float32)
    bec = np.asarray(bec, np.float32)
    per_core = []
    for r in range(NCORES):
        lo, hi = r * VS, (r + 1) * VS
        w0p = np.zeros((D, VSP), np.float32)
        w0p[:, :VS] = Wec[0:D, lo:hi]
        w1p = np.zeros((D, VSP), np.float32)
        w1p[:, :VS] = Wec[D:2 * D, lo:hi] * SW1
        bp = np.zeros((VSP,), np.float32)
        bp[:VS] = bec[lo:hi]
        per_core.append({
            "w0": np.ascontiguousarray(w0p.astype(bf)),
            "w18": np.ascontiguousarray(w1p.astype(f8)),
            "becp": np.ascontiguousarray(
                bp.reshape(NCHUNK, 128).T.astype(bf)),
        })
    return shared, per_core


def kernel(x, x_ids, Wq, bq, Wk, bk, Wv, bv, Wec, bec):
    shared, per_core = _host_inputs(x, x_ids, Wq, bq, Wk, bk, Wv, bv,
                                    Wec, bec)
    in_maps = [{**shared, **pc} for pc in per_core]

    nc = _get_program()
    from concourse.bass_utils import run_bass_kernel_spmd
    res = run_bass_kernel_spmd(nc, in_maps, core_ids=list(range(NCORES)))

    # gather raw exp(logits) shards -> (B, V) fp32
    outp = np.empty((B, V), np.float32)
    for r in range(NCORES):
        parts = []
        for g in range(4):
            o = np.asarray(res.results[r][f"out{g}"])
            parts.append(o.reshape(128, BANKS[g], B).transpose(2, 1, 0)
                         .reshape(B, BANKS[g] * 128).astype(np.float32))
        shard = np.concatenate(parts, axis=1)       # (B, VSP)
        outp[:, r * VS:(r + 1) * VS] = shard[:, :VS]

    # host epilogue: seen-id mask (O(B*S) scatter) + softmax normalize
    ids = np.asarray(x_ids).astype(np.int64)
    mask = (ids != 0) & (ids != 1)
    bidx = np.arange(B)[:, None]
    em = np.zeros((B, V), bool)
    em[np.broadcast_to(bidx, ids.shape)[mask], ids[mask]] = True
    outp[em] = 0.0
    gsum = outp.astype(np.float64).sum(axis=1)
    outp *= (1.0 / gsum)[:, None].astype(np.float32)
    return outp


# revision 54
# speedup vs baseline: 1.0527x; 1.0527x over previous
"""Trainium2 Bass kernel for nn_Explore_decoder_add (histogram_binning).

Strategy (8 NeuronCores, tensor-parallel on vocab), v9:
  - Wec split: W0 (h_t half) stays bf16 (precision-critical: h ~ N(0,1));
    W1 (c_s half) is fp8e4 x4 (its logit contribution is ~15x smaller).
    Measured end-to-end rel err ~5e-3 vs the 2e-2 gate.
  - The seen-id penalty (histogram) and the softmax normalization are
    applied on the HOST: the kernel returns raw exp(logits) per shard
    (fp16); the host zeroes the <=3200 masked entries, sums, normalizes.
  - Additive-attention bias fold: the host solves Wq^T d_b = k_b + bias
    (k_b = Wk^T x_b0 + bq + bk) and ships x' = x + d_b in fp16, so
    tanh(Wq^T x'_bs) == tanh(q + k) exactly.  The pooling q-path is then
    just 8 merged [128,400] fp16 matmuls + 8 constant-bias tanh
    activations over two batches each -- no per-batch bias plumbing on
    the critical path at all.
  - c_s still uses the ORIGINAL x (fp8 x16, s-on-partitions layout);
    scores -> fp8 exp -> PE-summed denominator (ones column holds 64 so
    v8 = csT/(64*ssum) = c_s_normalized/4, cancelling the x4 on W1).
  - bec is added mid-stream per PSUM bank by a DVE broadcast add
    (stride-0 free-dim AP) -- off the epilogue critical path.
  - DMA: sync ring carries packb/xqh/w0 g0..g3/w18 g0..g3 in consumption
    order (W1 groups last; pooling finishes before they land); gpsimd
    ring carries becp/xs in parallel.  Four per-bank fp16 output
    tensors, contiguous in DRAM, written from scalar/gpsimd/sync rings
    as each bank closes.
  - tile_wait_until hints push all bank matmuls after the pooling chain
    in the scheduler's static order (runtime stays semaphore-driven),
    keeping the tanh cadence ACT-bound instead of PE-bound.
"""

import numpy as np
import ml_dtypes

B, S, D = 16, 200, 128
V = 100000
NCORES = 8
VS = V // NCORES            # 12500 vocab per core
NCHUNK = 98                 # 98 chunks of 128
VSP = NCHUNK * 128          # 12544 padded shard width
BANKS = (25, 25, 25, 23)    # chunks per PSUM bank (sum = 98)
SX = 16.0                   # fp8 scale on x (c_s path)
SW1 = 4.0                   # fp8 scale on W1
SK = 64.0                   # ssum ones value (= SX * SW1)
XQ_COLS = D + B * S         # wq columns + x' columns in the xqh tensor

_prog_cache = {}


def _build_program():
    import concourse.bacc as bacc
    import concourse.mybir as mybir
    import concourse.tile as tile

    f32 = mybir.dt.float32
    f16 = mybir.dt.float16
    bf16 = mybir.dt.bfloat16
    f8 = mybir.dt.float8e4
    OP = mybir.AluOpType
    ACT = mybir.ActivationFunctionType

    nc = bacc.Bacc("TRN2", target_bir_lowering=False, debug=False,
                   num_devices=NCORES)

    # ---- I/O -------------------------------------------------------------
    packb = nc.dram_tensor("packb", (D, B + 1), bf16,
                           kind="ExternalInput").ap()
    xqh = nc.dram_tensor("xqh", (D, XQ_COLS), f16, kind="ExternalInput").ap()
    xs0 = nc.dram_tensor("xs0", (128, B, D), f8, kind="ExternalInput").ap()
    xs1 = nc.dram_tensor("xs1", (72, B, D), f8, kind="ExternalInput").ap()
    w0 = nc.dram_tensor("w0", (D, VSP), bf16, kind="ExternalInput").ap()
    w18 = nc.dram_tensor("w18", (D, VSP), f8, kind="ExternalInput").ap()
    becp = nc.dram_tensor("becp", (128, NCHUNK), bf16,
                          kind="ExternalInput").ap()
    outs = [nc.dram_tensor(f"out{g}", (128, BANKS[g] * B), f16,
                           kind="ExternalOutput").ap() for g in range(4)]

    with tile.TileContext(nc) as tc:
        with (
            tc.tile_pool(name="sb", bufs=1) as sb,
            tc.tile_pool(name="pq", bufs=1, space="PSUM") as pq,
            tc.tile_pool(name="pp", bufs=1, space="PSUM") as pp,
        ):
            # ---- sync ring: packb, xqh, then w0/w18 in bank order --------
            packb_sb = sb.tile([D, B + 1], bf16, name="packb_sb")
            nc.sync.dma_start(out=packb_sb[:, :], in_=packb[:, :])
            x0T_sb = packb_sb[:, 0:B]
            wv_sb = packb_sb[:, B:B + 1]

            xqh_sb = sb.tile([D, XQ_COLS], f16, name="xqh_sb")
            cuts = (0, 528, 1328, XQ_COLS)
            for i in range(3):
                nc.sync.dma_start(out=xqh_sb[:, cuts[i]:cuts[i + 1]],
                                  in_=xqh[:, cuts[i]:cuts[i + 1]])
            wqh_sb = xqh_sb[:, 0:D]

            w0_sb = sb.tile([D, VSP], bf16, name="w0_sb")
            w18_sb = sb.tile([D, VSP], f8, name="w18_sb")
            for g in range(4):
                c0 = sum(BANKS[:g]) * 128
                c1 = c0 + BANKS[g] * 128
                nc.sync.dma_start(out=w0_sb[:, c0:c1], in_=w0[:, c0:c1])
            for g in range(4):
                c0 = sum(BANKS[:g]) * 128
                c1 = c0 + BANKS[g] * 128
                nc.sync.dma_start(out=w18_sb[:, c0:c1], in_=w18[:, c0:c1])

            # ---- gpsimd ring: becp, xs (parallel with sync ring) ---------
            becp2_sb = sb.tile([128, NCHUNK], bf16, name="becp2_sb")
            nc.gpsimd.dma_start(out=becp2_sb[:, :], in_=becp[:, :])
            xs0_sb = sb.tile([128, B, D], f8, name="xs0_sb")
            nc.gpsimd.dma_start(out=xs0_sb[:, :, :], in_=xs0[:, :, :])
            xs1_sb = sb.tile([128, B, D], f8, name="xs1_sb")
            nc.gpsimd.dma_start(out=xs1_sb[0:72, :, :], in_=xs1[:, :, :])

            # ---- constants ----------------------------------------------
            ones64 = sb.tile([128, 1], f8, name="ones64")
            nc.gpsimd.memset(ones64[:, :], SK)
            ones_row = sb.tile([1, 128], f32, name="ones_row")
            nc.gpsimd.memset(ones_row[:, :], 1.0)

            # ---- pooling chain (critical path; emitted first) ------------
            pmiscA = pp.tile([128, 512], f32, name="pmiscA", tag="miscA")
            pmiscB = pp.tile([128, 512], f32, name="pmiscB", tag="miscB")
            fT = sb.tile([128, B, S], bf16, name="fT")
            scT0 = pmiscA[:, 0:B]
            scT1 = pmiscA[0:72, 2 * B:3 * B]
            qps = [pq.tile([128, 2, S], f32, name=f"qps{i}", tag=f"q{i}")
                   for i in range(2)]
            # W0 terms are emitted in ~12-chunk slices interleaved with the
            # q/tanh pipeline: keeps the PE duty cycle high (full p-state)
            # and finishes W0 by the end of pooling.
            ps = [pp.tile([128, 32, B], f32, name=f"ps{g}", tag=f"ps{g}")
                  for g in range(4)]

            def bank_of(c):
                t = 0
                for g in range(4):
                    if c < t + BANKS[g]:
                        return g, c - t, t
                    t += BANKS[g]
                raise AssertionError

            def emit_w0(c0, c1):
                for c in range(c0, min(c1, NCHUNK)):
                    g, cl, _ = bank_of(c)
                    nc.tensor.matmul(
                        out=ps[g][:, cl, :],
                        lhsT=w0_sb[:, c * 128:(c + 1) * 128],
                        rhs=x0T_sb, start=(cl == 0), stop=False)

            for g2 in range(8):
                b = 2 * g2
                tile_i = qps[g2 % 2]
                nc.tensor.matmul(
                    out=tile_i[:, :, :], lhsT=wqh_sb,
                    rhs=xqh_sb[:, D + b * S:D + (b + 2) * S],
                    start=True, stop=True)
                nc.scalar.activation(out=fT[:, b:b + 2, :],
                                     in_=tile_i[:, :, :], func=ACT.Tanh)
                emit_w0(g2 * 13, (g2 + 1) * 13)
            # mid-stream bec broadcast-adds (DVE), per closed-W0 bank
            t = 0
            for g in range(4):
                nb = BANKS[g]
                nc.vector.tensor_tensor(
                    out=ps[g][:, 0:nb, :], in0=ps[g][:, 0:nb, :],
                    in1=becp2_sb[:, t:t + nb].unsqueeze(2)
                        .broadcast_to([128, nb, B]),
                    op=OP.add)
                t += nb
            for b in range(B):
                nc.tensor.matmul(out=scT0[:, b:b + 1],
                                 lhsT=fT[:, b, 0:128], rhs=wv_sb,
                                 start=(b == 0), stop=(b == B - 1))
                nc.tensor.matmul(out=scT1[:, b:b + 1],
                                 lhsT=fT[:, b, 128:200], rhs=wv_sb,
                                 start=(b == 0), stop=(b == B - 1))
            e8_0 = sb.tile([128, B], f8, name="e8_0")
            nc.scalar.activation(out=e8_0[:, :], in_=scT0, func=ACT.Exp)
            e8_1 = sb.tile([128, B], f8, name="e8_1")
            nc.scalar.activation(out=e8_1[0:72, :], in_=scT1, func=ACT.Exp)
            ssum_ps = pmiscB[0:1, 0:B]
            nc.tensor.matmul(out=ssum_ps, lhsT=ones64[:, :],
                             rhs=e8_0[:, :], start=True, stop=False)
            nc.tensor.matmul(out=ssum_ps, lhsT=ones64[0:72, :],
                             rhs=e8_1[0:72, :], start=False, stop=True)
            sinv_row = sb.tile([1, B], f32, name="sinv_row")
            nc.vector.reciprocal(sinv_row[:, :], ssum_ps)

            csT = pmiscB[:, 2 * B:3 * B]
            for b in range(B):
                nc.tensor.matmul(out=csT[:, b:b + 1], lhsT=xs0_sb[:, b, :],
                                 rhs=e8_0[:, b:b + 1], start=(b == 0),
                                 stop=False)
                nc.tensor.matmul(out=csT[:, b:b + 1],
                                 lhsT=xs1_sb[0:72, b, :],
                                 rhs=e8_1[0:72, b:b + 1], start=False,
                                 stop=(b == B - 1))
            sinv_ps = pmiscB[:, 3 * B:4 * B]
            nc.tensor.matmul(out=sinv_ps, lhsT=ones_row[0:1, :],
                             rhs=sinv_row[:, :], start=True, stop=True)
            sinv_sb = sb.tile([128, B], f32, name="sinv_sb")
            nc.vector.tensor_copy(sinv_sb[:, :], sinv_ps)
            v8 = sb.tile([128, B], f8, name="v8")
            nc.vector.tensor_tensor(out=v8[:, :], in0=csT,
                                    in1=sinv_sb[:, :], op=OP.mult)

            # W1 terms close each bank; exp; per-bank output DMA.
            out_dma = [nc.sync, nc.gpsimd, nc.gpsimd, nc.sync]
            t = 0
            for g in range(4):
                nb = BANKS[g]
                with tc.tile_wait_until(0.21 + 0.0005 * g):
                    for cl in range(nb):
                        c = t + cl
                        nc.tensor.matmul(
                            out=ps[g][:, cl, :],
                            lhsT=w18_sb[:, c * 128:(c + 1) * 128],
                            rhs=v8[:, :], start=False,
                            stop=(cl == nb - 1))
                    exp_g = sb.tile([128, nb, B], f16, name=f"exp{g}")
                    nc.scalar.activation(out=exp_g[:, :, :],
                                         in_=ps[g][:, 0:nb, :],
                                         func=ACT.Exp)
                    out_dma[g].dma_start(
                        out=outs[g].rearrange("p (c b) -> p c b", b=B),
                        in_=exp_g[:, :, :])
                t += nb

    nc.compile()
    return nc


def _get_program():
    if "nc" not in _prog_cache:
        _prog_cache["nc"] = _build_program()
    return _prog_cache["nc"]


def _host_inputs(x, x_ids, Wq, bq, Wk, bk, Wv, bv, Wec, bec):
    """Shared + per-core input arrays (host re-encodes layouts and folds
    the additive-attention bias into x via a 128x128 solve)."""
    bf = ml_dtypes.bfloat16
    f8 = ml_dtypes.float8_e4m3
    x = np.asarray(x, dtype=np.float32)
    x8 = (x * SX).astype(f8)                       # (B,S,D), c_s path
    # fold k_b + bias into the q path: x' = x + Wq^-T (k_b + bq + bk)
    Wq64 = np.asarray(Wq, np.float64)
    k_host = (x[:, 0, :].astype(np.float64) @ np.asarray(Wk, np.float64)
              + np.asarray(bq, np.float64) + np.asarray(bk, np.float64))
    delta = np.linalg.solve(Wq64.T, k_host.T).T    # (B, D)
    xp = x + delta[:, None, :].astype(np.float32)  # (B,S,D)
    xq = np.empty((D, XQ_COLS), np.float16)
    xq[:, 0:D] = np.asarray(Wq, np.float32).astype(np.float16)
    xq[:, D:] = xp.transpose(2, 0, 1).reshape(D, B * S).astype(np.float16)
    packb = np.concatenate([
        np.ascontiguousarray(x[:, 0, :].T.astype(bf)),
        np.asarray(Wv, np.float32).astype(bf),
    ], axis=1)
    shared = {
        "packb": np.ascontiguousarray(packb),
        "xqh": np.ascontiguousarray(xq),
        "xs0": np.ascontiguousarray(x8[:, 0:128, :].transpose(1, 0, 2)),
        "xs1": np.ascontiguousarray(x8[:, 128:200, :].transpose(1, 0, 2)),
    }
    Wec = np.asarray(Wec, np.float32)
    bec = np.asarray(bec, np.float32)
    per_core = []
    for r in range(NCORES):
        lo, hi = r * VS, (r + 1) * VS
        w0p = np.zeros((D, VSP), np.float32)
        w0p[:, :VS] = Wec[0:D, lo:hi]
        w1p = np.zeros((D, VSP), np.float32)
        w1p[:, :VS] = Wec[D:2 * D, lo:hi] * SW1
        bp = np.zeros((VSP,), np.float32)
        bp[:VS] = bec[lo:hi]
        per_core.append({
            "w0": np.ascontiguousarray(w0p.astype(bf)),
            "w18": np.ascontiguousarray(w1p.astype(f8)),
            "becp": np.ascontiguousarray(
                bp.reshape(NCHUNK, 128).T.astype(bf)),
        })
    return shared, per_core


def kernel(x, x_ids, Wq, bq, Wk, bk, Wv, bv, Wec, bec):
    shared, per_core = _host_inputs(x, x_ids, Wq, bq, Wk, bk, Wv, bv,
                                    Wec, bec)
    in_maps = [{**shared, **pc} for pc in per_core]

    nc = _get_program()
    from concourse.bass_utils import run_bass_kernel_spmd
    res = run_bass_kernel_spmd(nc, in_maps, core_ids=list(range(NCORES)))

    # gather raw exp(logits) shards -> (B, V) fp32
    outp = np.empty((B, V), np.float32)
    for r in range(NCORES):
        parts = []
        for g in range(4):
            o = np.asarray(res.results[r][f"out{g}"])
            parts.append(o.reshape(128, BANKS[g], B).transpose(2, 1, 0)
                         .reshape(B, BANKS[g] * 128).astype(np.float32))
        shard = np.concatenate(parts, axis=1)       # (B, VSP)
        outp[:, r * VS:(r + 1) * VS] = shard[:, :VS]

    # host epilogue: seen-id mask (O(B*S) scatter) + softmax normalize
    ids = np.asarray(x_ids).astype(np.int64)
    mask = (ids != 0) & (ids != 1)
    bidx = np.arange(B)[:, None]
    em = np.zeros((B, V), bool)
    em[np.broadcast_to(bidx, ids.shape)[mask], ids[mask]] = True
    outp[em] = 0.0
    gsum = outp.astype(np.float64).sum(axis=1)
    outp *= (1.0 / gsum)[:, None].astype(np.float32)
    return outp
